# revision 26
# baseline (speedup 1.0000x reference)
"""GatedDeltaNet mixer on 8 Trainium2 NeuronCores.

Sharding: data-parallel over batch (cores 0-3 = batch 0, cores 4-7 = batch 1),
tensor-parallel over heads within each group (4 heads/core). Per core:
q/k/v/g/beta projections (bf16), causal depthwise conv (DVE shifted fused ops),
per-head LN (ones-matmul broadcast stats), chunked delta-rule scan (chunk=128,
decay matrices from softplus/cumsum matmuls + ACT exp), silu gating, AllGather
of the gated output within each 4-core group, then a column-sharded output
projection. Host only transposes/shards/casts inputs and concatenates outputs.
"""

import sys

sys.path.insert(0, "/opt/trn_rl_repo")

import ml_dtypes
import numpy as np

import concourse.bass as bass
import concourse.mybir as mybir
import concourse.tile as tile
from concourse.bass_utils import run_bass_kernel_spmd
from concourse.masks import make_identity, make_lower_triangular, make_upper_triangular
from concourse.tile_sem_assignment import N_PROCS
from concourse.vector_clock import ScopedClock, VectorClock

def _split_sync_waits_json(bir_json: bytes) -> bytes:
    """Legalize BIR sync waits for this container's walrus build.

    The walrus here encodes at most one sync-wait command on a regular
    instruction (two on EventSemaphore). Tile's sem-assignment attaches the
    full wait set to the consuming instruction, so spill the excess onto
    EventSemaphore carriers inserted just before it on the same engine —
    the engine executes serially, so the conjunction is preserved.
    """
    import orjson

    d = orjson.loads(bir_json)
    n = 0
    for func in d.get("functions", []):
        for bb in func.get("blocks", []):
            out = []
            for inst in bb.get("instructions", []):
                si = inst.get("sync_info")
                if si:
                    ws = si.get("on_wait") or []
                    cap = 2 if inst.get("opcode") == "EventSemaphore" else 1
                    if len(ws) > cap:
                        for w in ws[:-cap]:
                            n += 1
                            out.append({
                                "debug": inst.get("debug"),
                                "engine": inst["engine"],
                                "ins": [],
                                "name": f"SWS-{n}",
                                "opcode": "EventSemaphore",
                                "outs": [],
                                "sync_info": {"on_update": [], "on_wait": [w]},
                            })
                        si["on_wait"] = ws[-cap:]
                out.append(inst)
            bb["instructions"] = out
    return orjson.dumps(d)


def _install_wait_split_hook():
    import concourse.bass2jax as _b2j
    import concourse.bass_utils as _bu

    if getattr(_bu, "_wait_split_installed", False):
        return
    _orig = _bu.compile_bir_kernel

    def _patched(bir_json, tmpdir, neff_name="file.neff"):
        return _orig(_split_sync_waits_json(bir_json), tmpdir, neff_name)

    _bu.compile_bir_kernel = _patched
    _b2j.compile_bir_kernel = _patched
    _bu._wait_split_installed = True


_install_wait_split_hook()

BF16 = mybir.dt.bfloat16
F32 = mybir.dt.float32
AF = mybir.ActivationFunctionType
OP = mybir.AluOpType
BF = ml_dtypes.bfloat16

B, L, D = 2, 2048, 2048
H, DK, DV, K = 16, 128, 128, 4
CH = 128               # scan chunk length
NTC = L // CH          # 16 chunks
NK = D // 128          # 16 contraction tiles
HL = 4                 # heads per core
FSH = HL * DK          # 512 local feature columns
NCORES = 8
GROUPS = [[0, 1, 2, 3], [4, 5, 6, 7]]
EPS = 1e-5
NEG = -1.0e9


class _SplitDrainTC(tile.TileContext):
    """TileContext whose exit drain splits its semaphore waits.

    The walrus build here caps sync-wait commands at 1 per regular
    instruction; Tile's stock exit drain carries one wait per logical proc
    and fails to compile. Waits are moved onto a chain of NOPs instead.
    """

    def _drain_and_barrier(self, tick_clock, wait_clock):
        g = tick_clock.global_clock
        vals = [g[p] for p in range(N_PROCS)]
        for p in range(N_PROCS):
            if vals[p] <= 0:
                continue
            cvals = [vals[q] if q == p else 0 for q in range(N_PROCS)]
            d = self.nc.sync.nop(nofuse=True)
            wait_clock.add_sem_waits(d.ins, ScopedClock({None: VectorClock(cvals)}))
        self.nc.sync.drain()

        self.nc.all_engine_barrier()
        assert self.sems is not None
        popped = self.nc._tile_sem_poison_stack.pop()
        assert popped is self._sem_poison
        self.nc.clear_and_free_semaphores(list(self.sems.allocated().values()))
        self.nc.all_engine_barrier()


def build_kernel(reps: int = 1) -> bass.Bass:
    nc = bass.Bass()

    hT = nc.declare_dram_parameter("hT", [D, L], BF16, isOutput=False)
    wq = nc.declare_dram_parameter("wq", [D, FSH], BF16, isOutput=False)
    wk = nc.declare_dram_parameter("wk", [D, FSH], BF16, isOutput=False)
    wg = nc.declare_dram_parameter("wg", [D, FSH], BF16, isOutput=False)
    wv = nc.declare_dram_parameter("wv", [D, FSH], BF16, isOutput=False)
    wb = nc.declare_dram_parameter("wb", [D, HL], BF16, isOutput=False)
    wo = nc.declare_dram_parameter("wo", [H * DV, FSH], BF16, isOutput=False)
    qcw = nc.declare_dram_parameter("qcw", [128, HL * K], F32, isOutput=False)
    kcw = nc.declare_dram_parameter("kcw", [128, HL * K], F32, isOutput=False)
    qcb = nc.declare_dram_parameter("qcb", [128, HL], F32, isOutput=False)
    kcb = nc.declare_dram_parameter("kcb", [128, HL], F32, isOutput=False)
    qnw = nc.declare_dram_parameter("qnw", [128, 1], F32, isOutput=False)
    qnb = nc.declare_dram_parameter("qnb", [128, 1], F32, isOutput=False)
    knw = nc.declare_dram_parameter("knw", [128, 1], F32, isOutput=False)
    knb = nc.declare_dram_parameter("knb", [128, 1], F32, isOutput=False)
    bbb = nc.declare_dram_parameter("bbb", [128, HL], F32, isOutput=False)
    out = nc.declare_dram_parameter("out", [L, FSH], F32, isOutput=True)

    og_d = nc.dram_tensor("og_d", [FSH, L], BF16)
    og_all = [nc.dram_tensor(f"og_all{h}", [4 * 128, L], BF16) for h in range(HL)]
    og_dh = [nc.dram_tensor(f"og_dh{j}", [128, L // 2], BF16) for j in range(2)]
    og_ah = [nc.dram_tensor(f"og_ah{j}", [4 * 128, L // 2], BF16) for j in range(2)]

    with _SplitDrainTC(nc) as tc:
        with tc.tile_pool(name="ps", bufs=8, space="PSUM") as ps, \
             tc.tile_pool(name="cpool", bufs=1) as cpool:
            consts = _build_consts(nc, cpool)
            for _rep in range(reps):
                _build_main(nc, tc, (ps, ps), consts, locals())
    return nc


def _build_consts(nc, cpool):
    ones_sc = cpool.tile([128, 128], BF16, tag="ones_sc")
    nc.vector.memset(ones_sc, 1.0 / 128.0)
    ones_one = cpool.tile([128, 128], BF16, tag="ones_one")
    nc.vector.memset(ones_one, 1.0)
    negu = cpool.tile([128, 128], BF16, tag="negu")
    make_upper_triangular(nc, negu, val=-1.0, diag=True)
    ident = cpool.tile([128, 128], BF16, tag="ident")
    make_identity(nc, ident)
    maskc = cpool.tile([128, 128], BF16, tag="maskc")
    make_lower_triangular(nc, maskc, val=NEG, diag=False)
    eps_t = cpool.tile([128, 1], F32, tag="eps")
    nc.vector.memset(eps_t, EPS)
    return dict(ones_sc=ones_sc, ones_one=ones_one, negu=negu,
                ident=ident, maskc=maskc, eps_t=eps_t)


def _build_main(nc, tc, ps_pools, consts, t):
    ps, pss = ps_pools
    ones_sc, ones_one, negu = consts["ones_sc"], consts["ones_one"], consts["negu"]
    ident, maskc, eps_t = consts["ident"], consts["maskc"], consts["eps_t"]
    hT, wq, wk, wg, wv, wb = t["hT"], t["wq"], t["wk"], t["wg"], t["wv"], t["wb"]
    wo, qcw, kcw, qcb, kcb = t["wo"], t["qcw"], t["kcw"], t["qcb"], t["kcb"]
    qnw, qnb, knw, knb, bbb = t["qnw"], t["qnb"], t["knw"], t["knb"], t["bbb"]
    out, og_d, og_all = t["out"], t["og_d"], t["og_all"]
    og_dh, og_ah = t["og_dh"], t["og_ah"]

    with tc.tile_pool(name="wp", bufs=1) as wp:
        # ---- persistent smalls ------------------------------------------
        qcw_t = wp.tile([128, HL * K], F32, tag="qcw")
        nc.sync.dma_start(out=qcw_t, in_=qcw[:, :])
        kcw_t = wp.tile([128, HL * K], F32, tag="kcw")
        nc.sync.dma_start(out=kcw_t, in_=kcw[:, :])
        qcb_t = wp.tile([128, HL], F32, tag="qcb")
        nc.sync.dma_start(out=qcb_t, in_=qcb[:, :])
        kcb_t = wp.tile([128, HL], F32, tag="kcb")
        nc.sync.dma_start(out=kcb_t, in_=kcb[:, :])
        qnw_t = wp.tile([128, 1], F32, tag="qnw")
        nc.sync.dma_start(out=qnw_t, in_=qnw[:, :])
        qnb_t = wp.tile([128, 1], F32, tag="qnb")
        nc.sync.dma_start(out=qnb_t, in_=qnb[:, :])
        knw_t = wp.tile([128, 1], F32, tag="knw")
        nc.sync.dma_start(out=knw_t, in_=knw[:, :])
        knb_t = wp.tile([128, 1], F32, tag="knb")
        nc.sync.dma_start(out=knb_t, in_=knb[:, :])

        v_all = wp.tile([128, NTC * FSH], BF16, tag="v_all")
        sp_all = wp.tile([128, NTC * HL], F32, tag="sp_all")
        spn_all = wp.tile([128, NTC * HL], F32, tag="spn_all")

        hT_t = []  # populated inside the hT pool scope below

        with tc.tile_pool(name="wk1", bufs=2) as wk1, \
             tc.tile_pool(name="wk2", bufs=2) as wk2, \
             tc.tile_pool(name="wk3", bufs=3) as wk3, \
             tc.tile_pool(name="wk4", bufs=1) as wk4, \
             tc.tile_pool(name="wst", bufs=3) as wst:

            def prep_units(h):
                hsl = slice(h * 128, (h + 1) * 128)
                st = {}
                units = []

                def start_tensor(wparam, name):
                    def u():
                        wt = wst.tile([128, NK, 128], BF16, tag="wst")
                        nc.sync.dma_start(
                            out=wt,
                            in_=wparam[:, hsl].rearrange("(a p) b -> p a b", p=128))
                        st[name + "_w"] = wt
                        if name in ("q", "k"):
                            xpad = wk4.tile([128, 4 + L], BF16, tag="xpad")
                            nc.vector.memset(xpad[:, 0:4], 0.0)
                            st[name + "_xpad"] = xpad
                    return u

                def proj_tile(name, tt2, sink):
                    def u():
                        wt = st[name + "_w"]
                        pps = ps.tile([128, 512], F32, tag="ps")
                        tsl = slice(tt2 * 512, (tt2 + 1) * 512)
                        for kk in range(NK):
                            nc.tensor.matmul(pps, wt[:, kk, :], hT_t[kk][:, tsl],
                                             start=(kk == 0), stop=(kk == NK - 1))
                        sink(tt2, pps)
                    return u

                def xpad_sink(name):
                    def sink(tt2, pps):
                        xpad = st[name + "_xpad"]
                        nc.scalar.activation(
                            xpad[:, 4 + tt2 * 512 : 4 + (tt2 + 1) * 512],
                            pps, AF.Copy)
                    return sink

                def gsil_sink(tt2, pps):
                    if "gsil" not in st:
                        st["gsil"] = wk1.tile([128, L], BF16, tag="gsil", name="gsil")
                    nc.scalar.activation(
                        st["gsil"][:, tt2 * 512 : (tt2 + 1) * 512], pps, AF.Silu)

                def conv_unit(name, cw, cb):
                    def u():
                        xpad = st[name + "_xpad"]
                        y = wk2.tile([128, L], BF16, tag="convy")
                        nc.vector.tensor_scalar_mul(y, xpad[:, 1 : 1 + L],
                                                    cw[:, h * K : h * K + 1])
                        for s in (1, 2, 3):
                            nc.vector.scalar_tensor_tensor(
                                y, xpad[:, 1 + s : 1 + s + L],
                                cw[:, h * K + s : h * K + s + 1], y,
                                OP.mult, OP.add)
                        nc.scalar.activation(y, y, AF.Silu, bias=cb[:, h : h + 1])
                        st[name + "_sil"] = y
                    return u

                def ln_unit(name, dstname, tt2, lw, lb):
                    def u():
                        sil = st[name + "_sil"]
                        if dstname not in st:
                            st[dstname] = wk1.tile([128, L], BF16, tag=dstname,
                                                   name=dstname)
                        dst = st[dstname]
                        tsl = slice(tt2 * 512, (tt2 + 1) * 512)
                        sq = wk4.tile([128, 512], BF16, tag="sq")
                        nc.scalar.square(sq, sil[:, tsl])
                        mups = ps.tile([128, 512], F32, tag="ps")
                        nc.tensor.matmul(mups, ones_sc, sil[:, tsl],
                                         start=True, stop=True)
                        sqps = ps.tile([128, 512], F32, tag="ps")
                        nc.tensor.matmul(sqps, ones_sc, sq, start=True, stop=True)
                        m2 = wk4.tile([128, 512], F32, tag="m2")
                        nc.scalar.square(m2, mups)
                        vt = wk4.tile([128, 512], F32, tag="vt")
                        nc.vector.tensor_tensor(vt, sqps, m2, OP.subtract)
                        nc.scalar.activation(vt, vt, AF.Ln, bias=eps_t)
                        r0 = wk2.tile([128, 512], BF16, tag="r0")
                        nc.scalar.activation(r0, vt, AF.Exp, scale=-0.5)
                        r1 = wk2.tile([128, 512], BF16, tag="r1")
                        nc.vector.tensor_scalar_mul(r1, r0, lw)
                        s1 = wk2.tile([128, 512], BF16, tag="s1")
                        nc.vector.scalar_tensor_tensor(s1, mups, -1.0, r1,
                                                       OP.mult, OP.mult)
                        t1 = wk2.tile([128, 512], BF16, tag="t1")
                        nc.vector.tensor_tensor(t1, sil[:, tsl], r1, OP.mult)
                        nc.vector.scalar_tensor_tensor(dst[:, tsl], t1, lb, s1,
                                                       OP.add, OP.add)
                    return u

                def trans_unit():
                    # PE-mode transpose: dma_start_transpose would flip the
                    # DMA xbar mode, which Tile serializes against the
                    # collectives -- stalling every head behind the previous
                    # head's AllGather.
                    kln = st["kln"]
                    ktm = wk1.tile([128, L], BF16, tag="klntm")
                    for ci in range(NTC):
                        csl = slice(ci * 128, (ci + 1) * 128)
                        tps = ps.tile([128, 128], BF16, tag="ps")
                        nc.tensor.transpose(tps, kln[:, csl], ident)
                        nc.scalar.activation(ktm[:, csl], tps, AF.Copy)
                    st["kln_tm"] = ktm
                    sb = wk1.tile([128, 128], BF16, tag="sbf")
                    nc.vector.memset(sb, 0.0)
                    st["S_bf"] = sb
                    st["og_h"] = wk1.tile([128, L], BF16, tag="ogh", name="ogh")
                    st["ats_all"] = wk1.tile([128, L], BF16, tag="ats_all",
                                             name="ats_all")
                    st["qs_all"] = wk1.tile([128, L], BF16, tag="qs_all",
                                            name="qs_all")
                    st["kw_all"] = wk1.tile([128, L], BF16, tag="kw_all",
                                            name="kw_all")
                    st["pc_all"] = wk1.tile([128, NTC], F32, tag="pc_all",
                                            name="pc_all")

                units.append(start_tensor(wq, "q"))
                units += [proj_tile("q", t2, xpad_sink("q")) for t2 in range(4)]
                units.append(conv_unit("q", qcw_t, qcb_t))
                units.append(start_tensor(wk, "k"))
                units += [proj_tile("k", t2, xpad_sink("k")) for t2 in range(4)]
                units.append(conv_unit("k", kcw_t, kcb_t))
                units.append(start_tensor(wg, "g"))
                units += [proj_tile("g", t2, gsil_sink) for t2 in range(4)]
                units += [ln_unit("q", "qln", t2, qnw_t, qnb_t) for t2 in range(4)]
                units += [ln_unit("k", "kln", t2, knw_t, knb_t) for t2 in range(4)]
                units.append(trans_unit)
                units += [scan_a_chunk(h, st, ci) for ci in range(NTC)]
                return units, st

            def scan_a_chunk(h, st, ci):
                    def u():
                        qln, kln, kln_tm = st["qln"], st["kln"], st["kln_tm"]
                        csl = slice(ci * 128, (ci + 1) * 128)
                        spc = sp_all[:, ci * HL + h : ci * HL + h + 1]
                        spnc = spn_all[:, ci * HL + h : ci * HL + h + 1]

                        Yt = wk3.tile([128, 128], BF16, tag="Y")
                        nc.vector.tensor_scalar_mul(Yt, negu, spc)
                        gps = ps.tile([128, 128], F32, tag="ps")
                        nc.tensor.matmul(gps, ones_one, Yt, start=True, stop=False,
                                         skip_group_check=True)
                        gtps = ps.tile([128, 128], F32, tag="ps")
                        nc.tensor.matmul(gtps, Yt, ones_one, start=True, stop=True,
                                         skip_group_check=True)
                        ptile = wk3.tile([128, 128], F32, tag="pt")
                        nc.scalar.activation(ptile, gps, AF.Exp)
                        scol = wk3.tile([128, 1], F32, tag="scol")
                        nc.vector.scalar_tensor_tensor(scol, spnc, -1.0,
                                                       gtps[:, 0:1],
                                                       OP.mult, OP.subtract)
                        nc.tensor.matmul(gps, ident, maskc, start=False, stop=True,
                                         skip_group_check=True)
                        dexp = wk3.tile([128, 128], F32, tag="dexp")
                        nc.scalar.activation(dexp, gps, AF.Exp, bias=scol)

                        atps = ps.tile([128, 128], F32, tag="ps")
                        nc.tensor.matmul(atps, kln[:, csl], qln[:, csl],
                                         start=True, stop=True)
                        nc.vector.tensor_tensor(st["ats_all"][:, csl], atps, dexp,
                                                OP.mult)
                        nc.vector.tensor_tensor(st["qs_all"][:, csl], qln[:, csl],
                                                ptile, OP.mult)
                        nc.vector.tensor_scalar_mul(st["kw_all"][:, csl],
                                                    kln_tm[:, csl],
                                                    dexp[:, 127:128])
                        nc.vector.tensor_copy(st["pc_all"][:, ci : ci + 1],
                                              ptile[:, 127:128])
                    return u

            def scan_b_units(h, st, with_fin=True):
                """Serial state recurrence + gated output (no ACT work)."""
                units = []

                def chunk(ci):
                    def u():
                        S_bf, og_h, gsil = st["S_bf"], st["og_h"], st["gsil"]
                        csl = slice(ci * 128, (ci + 1) * 128)
                        vsl = slice(ci * FSH + h * 128, ci * FSH + (h + 1) * 128)

                        ops_ = ps.tile([128, 128], F32, tag="ps")
                        nc.tensor.matmul(ops_, v_all[:, vsl],
                                         st["ats_all"][:, csl],
                                         start=True, stop=False,
                                         skip_group_check=True)
                        nc.tensor.matmul(ops_, S_bf, st["qs_all"][:, csl],
                                         start=False, stop=True,
                                         skip_group_check=True)
                        nc.vector.tensor_tensor(og_h[:, csl], ops_, gsil[:, csl],
                                                OP.mult)

                        sd = wk3.tile([128, 128], BF16, tag="sd")
                        nc.vector.tensor_scalar_mul(sd, ident,
                                                    st["pc_all"][:, ci : ci + 1])
                        sps = ps.tile([128, 128], F32, tag="ps")
                        nc.tensor.matmul(sps, sd, S_bf, start=True, stop=False,
                                         skip_group_check=True)
                        nc.tensor.matmul(sps, st["kw_all"][:, csl], v_all[:, vsl],
                                         start=False, stop=True,
                                         skip_group_check=True)
                        nc.vector.tensor_copy(S_bf, sps)
                    return u

                units += [chunk(ci) for ci in range(NTC)]

                if with_fin:
                    def fin():
                        og_h = st["og_h"]
                        nc.sync.dma_start(out=og_d[h * 128 : (h + 1) * 128, :],
                                          in_=og_h)
                        nc.gpsimd.collective_compute(
                            "AllGather", OP.bypass, replica_groups=GROUPS,
                            ins=[og_d[h * 128 : (h + 1) * 128, :]],
                            outs=[og_all[h][:, :]],
                        )
                    units.append(fin)
                return units

            def emit_interleaved(a, b):
                na, nb = len(a), len(b)
                ia = ib = 0
                while ia < na or ib < nb:
                    if ib < nb and (ia >= na or ib * na <= ia * nb):
                        b[ib]()
                        ib += 1
                    else:
                        a[ia]()
                        ia += 1

            with tc.tile_pool(name="hTp", bufs=1) as hTp:
                for i in range(NK):
                    tt = hTp.tile([128, L], BF16, tag=f"hT{i}")
                    nc.sync.dma_start(out=tt, in_=hT[i * 128 : (i + 1) * 128, :])
                    hT_t.append(tt)

                # ---- phase 1: v (time-major) + beta softplus ------------
                with tc.tile_pool(name="vp", bufs=1) as vp, \
                     tc.tile_pool(name="vp2", bufs=2) as vp2:
                    wv_t = []
                    for i in range(NK):
                        tt = vp.tile([128, FSH], BF16, tag=f"wv{i}")
                        nc.sync.dma_start(out=tt, in_=wv[i * 128 : (i + 1) * 128, :])
                        wv_t.append(tt)
                    wb_t = []
                    for i in range(NK):
                        tt = vp.tile([128, HL], BF16, tag=f"wb{i}")
                        nc.sync.dma_start(out=tt, in_=wb[i * 128 : (i + 1) * 128, :])
                        wb_t.append(tt)
                    bbb_t = vp.tile([128, HL], F32, tag="bbb")
                    nc.sync.dma_start(out=bbb_t, in_=bbb[:, :])

                    for ci in range(NTC):
                        csl = slice(ci * 128, (ci + 1) * 128)
                        vps = ps.tile([128, FSH], F32, tag="ps")
                        for kk in range(NK):
                            nc.tensor.matmul(vps, hT_t[kk][:, csl], wv_t[kk],
                                             start=(kk == 0), stop=(kk == NK - 1))
                        nc.scalar.activation(v_all[:, ci * FSH : (ci + 1) * FSH],
                                             vps, AF.Copy)
                        bps = ps.tile([128, HL], F32, tag="ps")
                        for kk in range(NK):
                            nc.tensor.matmul(bps, hT_t[kk][:, csl], wb_t[kk],
                                             start=(kk == 0), stop=(kk == NK - 1))
                        zb = vp2.tile([128, HL], F32, tag="zb")
                        nc.vector.tensor_tensor(zb, bps, bbb_t, OP.add)
                        bsl = slice(ci * HL, (ci + 1) * HL)
                        # softplus(z) = ln(1 + exp(z))
                        e1 = vp2.tile([128, HL], F32, tag="e1")
                        nc.scalar.activation(e1, zb, AF.Exp)
                        nc.scalar.activation(sp_all[:, bsl], e1, AF.Ln, bias=1.0)
                        e2 = vp2.tile([128, HL], F32, tag="e2")
                        nc.scalar.activation(e2, zb, AF.Exp, scale=-1.0)
                        nc.scalar.activation(spn_all[:, bsl], e2, AF.Ln, bias=1.0)

                # ---- phase 2: pipelined heads (prep+scanA braided with
                #      previous head's scanB) -----------------------------
                prev = None
                for h in range(HL):
                    pu, st = prep_units(h)
                    su = scan_b_units(h - 1, prev) if prev is not None else []
                    emit_interleaved(pu, su)
                    prev = st
            # hT pool closed: its SBUF is reused by the out-projection tiles

            # ---- tail: last head's scanB + AllGather halves + out proj --
            with tc.tile_pool(name="p4w", bufs=1) as p4w, \
                 tc.tile_pool(name="p4s", bufs=20) as p4s, \
                 tc.tile_pool(name="p4o", bufs=2) as p4o:
                lh = HL - 1
                b3 = scan_b_units(lh, prev, with_fin=False)

                def ag_half(half):
                    def u():
                        hsl2 = slice(half * (L // 2), (half + 1) * (L // 2))
                        nc.sync.dma_start(out=og_dh[half][:, :],
                                          in_=prev["og_h"][:, hsl2])
                        nc.gpsimd.collective_compute(
                            "AllGather", OP.bypass, replica_groups=GROUPS,
                            ins=[og_dh[half][:, :]],
                            outs=[og_ah[half][:, :]],
                        )
                    return u

                b3 = b3[:8] + [ag_half(0)] + b3[8:] + [ag_half(1)]

                wo_t = []

                def wo_load():
                    for i in range(H * DV // 128):
                        tt = p4w.tile([128, FSH], BF16, tag=f"wo{i}")
                        nc.sync.dma_start(out=tt, in_=wo[i * 128 : (i + 1) * 128, :])
                        wo_t.append(tt)

                ogt = {}

                def og_src(ff, b):
                    hs, r = ff // 4, ff % 4
                    rsl = slice(r * 128, (r + 1) * 128)
                    bsl = slice(b * 512, (b + 1) * 512)
                    if hs < HL - 1:
                        return og_all[hs][rsl, bsl]
                    half, bb = (0, b) if b < 2 else (1, b - 2)
                    return og_ah[half][rsl, bb * 512 : (bb + 1) * 512]

                def load_unit(b, ffs):
                    def u():
                        for ff in ffs:
                            s = p4s.tile([128, 512], BF16, tag="ogs", name="ogs")
                            nc.sync.dma_start(out=s, in_=og_src(ff, b))
                            ogt[(b, ff)] = s
                    return u

                def mm_unit(b):
                    def u():
                        for tcl in range(4):
                            tci = b * 4 + tcl
                            ops_ = ps.tile([128, FSH], F32, tag="ps")
                            for ff in range(H * DV // 128):
                                nc.tensor.matmul(
                                    ops_, ogt[(b, ff)][:, tcl * 128 : (tcl + 1) * 128],
                                    wo_t[ff], start=(ff == 0),
                                    stop=(ff == H * DV // 128 - 1))
                            outt = p4o.tile([128, FSH], F32, tag="outt")
                            nc.scalar.activation(outt, ops_, AF.Copy)
                            nc.sync.dma_start(out=out[tci * 128 : (tci + 1) * 128, :],
                                              in_=outt)
                    return u

                hs012 = [ff for ff in range(H * DV // 128) if ff // 4 < HL - 1]
                hs3 = [ff for ff in range(H * DV // 128) if ff // 4 == HL - 1]
                emit_interleaved([wo_load, load_unit(0, hs012)], b3)
                load_unit(0, hs3)()
                load_unit(1, hs012)()
                mm_unit(0)()
                load_unit(1, hs3)()
                load_unit(2, hs012)()
                mm_unit(1)()
                load_unit(2, hs3)()
                load_unit(3, hs012)()
                mm_unit(2)()
                load_unit(3, hs3)()
                mm_unit(3)()


def _wo_reordered(Wo, cols):
    """Row blocks ordered to match per-head-slot AllGather layout:
    block (hs*4 + rank) = rows of global head (4*rank + hs)."""
    blocks = [Wo[(4 * r + hs) * 128 : (4 * r + hs + 1) * 128, cols]
              for hs in range(HL) for r in range(4)]
    return np.ascontiguousarray(np.concatenate(blocks, axis=0)).astype(BF)


def prep_inputs(inputs):
    """Shard/transpose/cast full inputs into 8 per-core in_maps."""
    f = {k: np.asarray(v, dtype=np.float32) for k, v in inputs.items()}
    h = f["hidden_states"]

    hT_b = [np.ascontiguousarray(h[b].T).astype(BF) for b in range(B)]
    in_maps = []
    for c in range(NCORES):
        b, r = c // 4, c % 4
        cols = slice(r * FSH, (r + 1) * FSH)

        def convw(w):
            m = w[cols].reshape(HL, 128, K).transpose(1, 0, 2).reshape(128, HL * K)
            return np.ascontiguousarray(m).astype(np.float32)

        def convb(bias):
            return np.ascontiguousarray(bias[cols].reshape(HL, 128).T).astype(np.float32)

        m = {
            "hT": hT_b[b],
            "wq": np.ascontiguousarray(f["Wq"][:, cols]).astype(BF),
            "wk": np.ascontiguousarray(f["Wk"][:, cols]).astype(BF),
            "wg": np.ascontiguousarray(f["Wg"][:, cols]).astype(BF),
            "wv": np.ascontiguousarray(f["Wv"][:, cols]).astype(BF),
            "wb": np.ascontiguousarray(f["Wb"][:, 4 * r : 4 * r + 4]).astype(BF),
            "wo": _wo_reordered(f["Wo"], cols),
            "qcw": convw(f["qconv_w"]),
            "kcw": convw(f["kconv_w"]),
            "qcb": convb(f["qconv_b"]),
            "kcb": convb(f["kconv_b"]),
            "qnw": f["qn_w"].reshape(128, 1).astype(np.float32),
            "qnb": f["qn_b"].reshape(128, 1).astype(np.float32),
            "knw": f["kn_w"].reshape(128, 1).astype(np.float32),
            "knb": f["kn_b"].reshape(128, 1).astype(np.float32),
            "bbb": np.ascontiguousarray(
                np.broadcast_to(f["bb"][4 * r : 4 * r + 4], (128, HL))
            ).astype(np.float32),
        }
        in_maps.append(m)
    return in_maps


_NC_CACHE = {}


def get_nc():
    if "nc" not in _NC_CACHE:
        _NC_CACHE["nc"] = build_kernel()
    return _NC_CACHE["nc"]


def assemble(results):
    full = np.empty((B, L, D), np.float32)
    for c in range(NCORES):
        b, r = c // 4, c % 4
        full[b][:, r * FSH : (r + 1) * FSH] = results[c]["out"]
    return full


def kernel(**inputs) -> np.ndarray:
    nc = get_nc()
    in_maps = prep_inputs(inputs)
    res = run_bass_kernel_spmd(nc, in_maps, list(range(NCORES)))
    return assemble(res.results)


# revision 31
# speedup vs baseline: 1.0924x; 1.0924x over previous
"""GatedDeltaNet mixer on 8 Trainium2 NeuronCores.

Sharding: data-parallel over batch (cores 0-3 = batch 0, cores 4-7 = batch 1),
tensor-parallel over heads within each group (4 heads/core). Per core:
q/k/v/g/beta projections (bf16), causal depthwise conv (DVE shifted fused ops),
per-head LN (ones-matmul broadcast stats), chunked delta-rule scan (chunk=128,
decay matrices from softplus/cumsum matmuls + ACT exp), silu gating, AllGather
of the gated output within each 4-core group, then a column-sharded output
projection. Host only transposes/shards/casts inputs and concatenates outputs.
"""

import sys

sys.path.insert(0, "/opt/trn_rl_repo")

import ml_dtypes
import numpy as np

import concourse.bass as bass
import concourse.mybir as mybir
import concourse.tile as tile
from concourse.bass_utils import run_bass_kernel_spmd
from concourse.masks import make_identity, make_lower_triangular, make_upper_triangular
from concourse.tile_sem_assignment import N_PROCS
from concourse.vector_clock import ScopedClock, VectorClock

def _split_sync_waits_json(bir_json: bytes) -> bytes:
    """Legalize BIR sync waits for this container's walrus build.

    The walrus here encodes at most one sync-wait command on a regular
    instruction (two on EventSemaphore). Tile's sem-assignment attaches the
    full wait set to the consuming instruction, so spill the excess onto
    EventSemaphore carriers inserted just before it on the same engine —
    the engine executes serially, so the conjunction is preserved.
    """
    import orjson

    d = orjson.loads(bir_json)
    n = 0
    for func in d.get("functions", []):
        for bb in func.get("blocks", []):
            out = []
            for inst in bb.get("instructions", []):
                si = inst.get("sync_info")
                if si:
                    ws = si.get("on_wait") or []
                    cap = 2 if inst.get("opcode") == "EventSemaphore" else 1
                    if len(ws) > cap:
                        for w in ws[:-cap]:
                            n += 1
                            out.append({
                                "debug": inst.get("debug"),
                                "engine": inst["engine"],
                                "ins": [],
                                "name": f"SWS-{n}",
                                "opcode": "EventSemaphore",
                                "outs": [],
                                "sync_info": {"on_update": [], "on_wait": [w]},
                            })
                        si["on_wait"] = ws[-cap:]
                out.append(inst)
            bb["instructions"] = out
    return orjson.dumps(d)


def _install_wait_split_hook():
    import concourse.bass2jax as _b2j
    import concourse.bass_utils as _bu

    if getattr(_bu, "_wait_split_installed", False):
        return
    _orig = _bu.compile_bir_kernel

    def _patched(bir_json, tmpdir, neff_name="file.neff"):
        return _orig(_split_sync_waits_json(bir_json), tmpdir, neff_name)

    _bu.compile_bir_kernel = _patched
    _b2j.compile_bir_kernel = _patched
    _bu._wait_split_installed = True


_install_wait_split_hook()

BF16 = mybir.dt.bfloat16
F32 = mybir.dt.float32
AF = mybir.ActivationFunctionType
OP = mybir.AluOpType
BF = ml_dtypes.bfloat16

B, L, D = 2, 2048, 2048
H, DK, DV, K = 16, 128, 128, 4
CH = 128               # scan chunk length
NTC = L // CH          # 16 chunks
NK = D // 128          # 16 contraction tiles
HL = 4                 # heads per core
FSH = HL * DK          # 512 local feature columns
NCORES = 8
GROUPS = [[0, 1, 2, 3], [4, 5, 6, 7]]
EPS = 1e-5
NEG = -1.0e9


class _SplitDrainTC(tile.TileContext):
    """TileContext whose exit drain splits its semaphore waits.

    The walrus build here caps sync-wait commands at 1 per regular
    instruction; Tile's stock exit drain carries one wait per logical proc
    and fails to compile. Waits are moved onto a chain of NOPs instead.
    """

    def _drain_and_barrier(self, tick_clock, wait_clock):
        g = tick_clock.global_clock
        vals = [g[p] for p in range(N_PROCS)]
        for p in range(N_PROCS):
            if vals[p] <= 0:
                continue
            cvals = [vals[q] if q == p else 0 for q in range(N_PROCS)]
            d = self.nc.sync.nop(nofuse=True)
            wait_clock.add_sem_waits(d.ins, ScopedClock({None: VectorClock(cvals)}))
        self.nc.sync.drain()

        self.nc.all_engine_barrier()
        assert self.sems is not None
        popped = self.nc._tile_sem_poison_stack.pop()
        assert popped is self._sem_poison
        self.nc.clear_and_free_semaphores(list(self.sems.allocated().values()))
        self.nc.all_engine_barrier()


def build_kernel(reps: int = 1) -> bass.Bass:
    nc = bass.Bass()

    hT = nc.declare_dram_parameter("hT", [D, L], BF16, isOutput=False)
    wq = nc.declare_dram_parameter("wq", [D, FSH], BF16, isOutput=False)
    wk = nc.declare_dram_parameter("wk", [D, FSH], BF16, isOutput=False)
    wg = nc.declare_dram_parameter("wg", [D, FSH], BF16, isOutput=False)
    wv = nc.declare_dram_parameter("wv", [D, FSH], BF16, isOutput=False)
    wb = nc.declare_dram_parameter("wb", [D, HL], BF16, isOutput=False)
    wo = nc.declare_dram_parameter("wo", [H * DV, FSH], BF16, isOutput=False)
    qcw = nc.declare_dram_parameter("qcw", [128, HL * K], F32, isOutput=False)
    kcw = nc.declare_dram_parameter("kcw", [128, HL * K], F32, isOutput=False)
    qcb = nc.declare_dram_parameter("qcb", [128, HL], F32, isOutput=False)
    kcb = nc.declare_dram_parameter("kcb", [128, HL], F32, isOutput=False)
    qnw = nc.declare_dram_parameter("qnw", [128, 1], F32, isOutput=False)
    qnb = nc.declare_dram_parameter("qnb", [128, 1], F32, isOutput=False)
    knw = nc.declare_dram_parameter("knw", [128, 1], F32, isOutput=False)
    knb = nc.declare_dram_parameter("knb", [128, 1], F32, isOutput=False)
    bbb = nc.declare_dram_parameter("bbb", [128, HL], F32, isOutput=False)
    out = nc.declare_dram_parameter("out", [L, FSH], F32, isOutput=True)

    og_d = nc.dram_tensor("og_d", [FSH, L], BF16)
    og_all = [nc.dram_tensor(f"og_all{h}", [4 * 128, L], BF16) for h in range(HL)]
    og_dh = [nc.dram_tensor(f"og_dh{j}", [128, L // 2], BF16) for j in range(2)]
    og_ah = [nc.dram_tensor(f"og_ah{j}", [4 * 128, L // 2], BF16) for j in range(2)]

    with _SplitDrainTC(nc) as tc:
        with tc.tile_pool(name="ps", bufs=8, space="PSUM") as ps, \
             tc.tile_pool(name="cpool", bufs=1) as cpool:
            consts = _build_consts(nc, cpool)
            for _rep in range(reps):
                _build_main(nc, tc, (ps, ps), consts, locals())
    return nc


def _build_consts(nc, cpool):
    ones_sc = cpool.tile([128, 128], BF16, tag="ones_sc")
    nc.vector.memset(ones_sc, 1.0 / 128.0)
    ones_one = cpool.tile([128, 128], BF16, tag="ones_one")
    nc.vector.memset(ones_one, 1.0)
    negu = cpool.tile([128, 128], BF16, tag="negu")
    make_upper_triangular(nc, negu, val=-1.0, diag=True)
    ident = cpool.tile([128, 128], BF16, tag="ident")
    make_identity(nc, ident)
    maskc = cpool.tile([128, 128], BF16, tag="maskc")
    make_lower_triangular(nc, maskc, val=NEG, diag=False)
    eps_t = cpool.tile([128, 1], F32, tag="eps")
    nc.vector.memset(eps_t, EPS)
    return dict(ones_sc=ones_sc, ones_one=ones_one, negu=negu,
                ident=ident, maskc=maskc, eps_t=eps_t)


def _build_main(nc, tc, ps_pools, consts, t):
    ps, pss = ps_pools
    ones_sc, ones_one, negu = consts["ones_sc"], consts["ones_one"], consts["negu"]
    ident, maskc, eps_t = consts["ident"], consts["maskc"], consts["eps_t"]
    hT, wq, wk, wg, wv, wb = t["hT"], t["wq"], t["wk"], t["wg"], t["wv"], t["wb"]
    wo, qcw, kcw, qcb, kcb = t["wo"], t["qcw"], t["kcw"], t["qcb"], t["kcb"]
    qnw, qnb, knw, knb, bbb = t["qnw"], t["qnb"], t["knw"], t["knb"], t["bbb"]
    out, og_d, og_all = t["out"], t["og_d"], t["og_all"]
    og_dh, og_ah = t["og_dh"], t["og_ah"]

    with tc.tile_pool(name="wp", bufs=1) as wp:
        # ---- persistent smalls ------------------------------------------
        qcw_t = wp.tile([128, HL * K], F32, tag="qcw")
        nc.sync.dma_start(out=qcw_t, in_=qcw[:, :])
        kcw_t = wp.tile([128, HL * K], F32, tag="kcw")
        nc.sync.dma_start(out=kcw_t, in_=kcw[:, :])
        qcb_t = wp.tile([128, HL], F32, tag="qcb")
        nc.sync.dma_start(out=qcb_t, in_=qcb[:, :])
        kcb_t = wp.tile([128, HL], F32, tag="kcb")
        nc.sync.dma_start(out=kcb_t, in_=kcb[:, :])
        qnw_t = wp.tile([128, 1], F32, tag="qnw")
        nc.sync.dma_start(out=qnw_t, in_=qnw[:, :])
        qnb_t = wp.tile([128, 1], F32, tag="qnb")
        nc.sync.dma_start(out=qnb_t, in_=qnb[:, :])
        knw_t = wp.tile([128, 1], F32, tag="knw")
        nc.sync.dma_start(out=knw_t, in_=knw[:, :])
        knb_t = wp.tile([128, 1], F32, tag="knb")
        nc.sync.dma_start(out=knb_t, in_=knb[:, :])

        v_all = wp.tile([128, NTC * FSH], BF16, tag="v_all")
        sp_all = wp.tile([128, NTC * HL], F32, tag="sp_all")
        spn_all = wp.tile([128, NTC * HL], F32, tag="spn_all")

        hT_t = []  # populated inside the hT pool scope below

        with tc.tile_pool(name="wk1", bufs=2) as wk1, \
             tc.tile_pool(name="wk2", bufs=2) as wk2, \
             tc.tile_pool(name="wk3", bufs=3) as wk3, \
             tc.tile_pool(name="wk4", bufs=1) as wk4, \
             tc.tile_pool(name="wst", bufs=3) as wst:

            def prep_units(h):
                hsl = slice(h * 128, (h + 1) * 128)
                st = {}
                units = []

                def start_tensor(wparam, name):
                    def u():
                        wt = wst.tile([128, NK, 128], BF16, tag="wst")
                        nc.sync.dma_start(
                            out=wt,
                            in_=wparam[:, hsl].rearrange("(a p) b -> p a b", p=128))
                        st[name + "_w"] = wt
                        if name in ("q", "k"):
                            xpad = wk4.tile([128, 4 + L], BF16, tag="xpad")
                            nc.vector.memset(xpad[:, 0:4], 0.0)
                            st[name + "_xpad"] = xpad
                    return u

                def proj_tile(name, tt2, sink):
                    def u():
                        wt = st[name + "_w"]
                        pps = ps.tile([128, 512], F32, tag="ps")
                        tsl = slice(tt2 * 512, (tt2 + 1) * 512)
                        for kk in range(NK):
                            nc.tensor.matmul(pps, wt[:, kk, :], hT_t[kk][:, tsl],
                                             start=(kk == 0), stop=(kk == NK - 1))
                        sink(tt2, pps)
                    return u

                def xpad_sink(name):
                    def sink(tt2, pps):
                        xpad = st[name + "_xpad"]
                        nc.scalar.activation(
                            xpad[:, 4 + tt2 * 512 : 4 + (tt2 + 1) * 512],
                            pps, AF.Copy)
                    return sink

                def gsil_sink(tt2, pps):
                    if "gsil" not in st:
                        st["gsil"] = wk1.tile([128, L], BF16, tag="gsil", name="gsil")
                    nc.scalar.activation(
                        st["gsil"][:, tt2 * 512 : (tt2 + 1) * 512], pps, AF.Silu)

                def conv_unit(name, cw, cb):
                    def u():
                        xpad = st[name + "_xpad"]
                        y = wk2.tile([128, L], BF16, tag="convy")
                        nc.vector.tensor_scalar_mul(y, xpad[:, 1 : 1 + L],
                                                    cw[:, h * K : h * K + 1])
                        for s in (1, 2, 3):
                            nc.vector.scalar_tensor_tensor(
                                y, xpad[:, 1 + s : 1 + s + L],
                                cw[:, h * K + s : h * K + s + 1], y,
                                OP.mult, OP.add)
                        nc.scalar.activation(y, y, AF.Silu, bias=cb[:, h : h + 1])
                        st[name + "_sil"] = y
                    return u

                def ln_unit(name, dstname, tt2, lw, lb):
                    def u():
                        sil = st[name + "_sil"]
                        if dstname not in st:
                            st[dstname] = wk1.tile([128, L], BF16, tag=dstname,
                                                   name=dstname)
                        dst = st[dstname]
                        tsl = slice(tt2 * 512, (tt2 + 1) * 512)
                        sq = wk4.tile([128, 512], BF16, tag="sq")
                        nc.scalar.square(sq, sil[:, tsl])
                        mups = ps.tile([128, 512], F32, tag="ps")
                        nc.tensor.matmul(mups, ones_sc, sil[:, tsl],
                                         start=True, stop=True)
                        sqps = ps.tile([128, 512], F32, tag="ps")
                        nc.tensor.matmul(sqps, ones_sc, sq, start=True, stop=True)
                        m2 = wk4.tile([128, 512], F32, tag="m2")
                        nc.scalar.square(m2, mups)
                        vt = wk4.tile([128, 512], F32, tag="vt")
                        nc.vector.tensor_tensor(vt, sqps, m2, OP.subtract)
                        nc.scalar.activation(vt, vt, AF.Ln, bias=eps_t)
                        r0 = wk2.tile([128, 512], BF16, tag="r0")
                        nc.scalar.activation(r0, vt, AF.Exp, scale=-0.5)
                        r1 = wk2.tile([128, 512], BF16, tag="r1")
                        nc.vector.tensor_scalar_mul(r1, r0, lw)
                        s1 = wk2.tile([128, 512], BF16, tag="s1")
                        nc.vector.scalar_tensor_tensor(s1, mups, -1.0, r1,
                                                       OP.mult, OP.mult)
                        t1 = wk2.tile([128, 512], BF16, tag="t1")
                        nc.vector.tensor_tensor(t1, sil[:, tsl], r1, OP.mult)
                        nc.vector.scalar_tensor_tensor(dst[:, tsl], t1, lb, s1,
                                                       OP.add, OP.add)
                    return u

                def trans_unit():
                    # PE-mode transpose: dma_start_transpose would flip the
                    # DMA xbar mode, which Tile serializes against the
                    # collectives -- stalling every head behind the previous
                    # head's AllGather.
                    kln = st["kln"]
                    ktm = wk1.tile([128, L], BF16, tag="klntm")
                    for ci in range(NTC):
                        csl = slice(ci * 128, (ci + 1) * 128)
                        tps = ps.tile([128, 128], BF16, tag="ps")
                        nc.tensor.transpose(tps, kln[:, csl], ident)
                        nc.scalar.activation(ktm[:, csl], tps, AF.Copy)
                    st["kln_tm"] = ktm
                    sb = wk1.tile([128, 128], BF16, tag="sbf")
                    nc.vector.memset(sb, 0.0)
                    st["S_bf"] = sb
                    st["og_h"] = wk1.tile([128, L], BF16, tag="ogh", name="ogh")
                    st["ats_all"] = wk1.tile([128, L], BF16, tag="ats_all",
                                             name="ats_all")
                    st["qs_all"] = wk1.tile([128, L], BF16, tag="qs_all",
                                            name="qs_all")
                    st["kw_all"] = wk1.tile([128, L], BF16, tag="kw_all",
                                            name="kw_all")
                    st["pc_all"] = wk1.tile([128, NTC], F32, tag="pc_all",
                                            name="pc_all")

                units.append(start_tensor(wq, "q"))
                units += [proj_tile("q", t2, xpad_sink("q")) for t2 in range(4)]
                units.append(conv_unit("q", qcw_t, qcb_t))
                units.append(start_tensor(wk, "k"))
                units += [proj_tile("k", t2, xpad_sink("k")) for t2 in range(4)]
                units.append(conv_unit("k", kcw_t, kcb_t))
                units.append(start_tensor(wg, "g"))
                units += [proj_tile("g", t2, gsil_sink) for t2 in range(4)]
                units += [ln_unit("q", "qln", t2, qnw_t, qnb_t) for t2 in range(4)]
                units += [ln_unit("k", "kln", t2, knw_t, knb_t) for t2 in range(4)]
                units.append(trans_unit)
                units += [scan_a_chunk(h, st, ci) for ci in range(NTC)]
                return units, st

            def scan_a_chunk(h, st, ci):
                    def u():
                        qln, kln, kln_tm = st["qln"], st["kln"], st["kln_tm"]
                        csl = slice(ci * 128, (ci + 1) * 128)
                        spc = sp_all[:, ci * HL + h : ci * HL + h + 1]
                        spnc = spn_all[:, ci * HL + h : ci * HL + h + 1]

                        Yt = wk3.tile([128, 128], BF16, tag="Y")
                        nc.vector.tensor_scalar_mul(Yt, negu, spc)
                        gps = ps.tile([128, 128], F32, tag="ps")
                        nc.tensor.matmul(gps, ones_one, Yt, start=True, stop=False,
                                         skip_group_check=True)
                        gtps = ps.tile([128, 128], F32, tag="ps")
                        nc.tensor.matmul(gtps, Yt, ones_one, start=True, stop=True,
                                         skip_group_check=True)
                        ptile = wk3.tile([128, 128], F32, tag="pt")
                        nc.scalar.activation(ptile, gps, AF.Exp)
                        scol = wk3.tile([128, 1], F32, tag="scol")
                        nc.vector.scalar_tensor_tensor(scol, spnc, -1.0,
                                                       gtps[:, 0:1],
                                                       OP.mult, OP.subtract)
                        nc.tensor.matmul(gps, ident, maskc, start=False, stop=True,
                                         skip_group_check=True)
                        dexp = wk3.tile([128, 128], F32, tag="dexp")
                        nc.scalar.activation(dexp, gps, AF.Exp, bias=scol)

                        atps = ps.tile([128, 128], F32, tag="ps")
                        nc.tensor.matmul(atps, kln[:, csl], qln[:, csl],
                                         start=True, stop=True)
                        nc.vector.tensor_tensor(st["ats_all"][:, csl], atps, dexp,
                                                OP.mult)
                        nc.vector.tensor_tensor(st["qs_all"][:, csl], qln[:, csl],
                                                ptile, OP.mult)
                        nc.vector.tensor_scalar_mul(st["kw_all"][:, csl],
                                                    kln_tm[:, csl],
                                                    dexp[:, 127:128])
                        nc.vector.tensor_copy(st["pc_all"][:, ci : ci + 1],
                                              ptile[:, 127:128])
                    return u

            def scan_b_units(h, st, with_fin=True):
                """Serial state recurrence + gated output (no ACT work)."""
                units = []

                def chunk(ci):
                    def u():
                        S_bf, og_h, gsil = st["S_bf"], st["og_h"], st["gsil"]
                        csl = slice(ci * 128, (ci + 1) * 128)
                        vsl = slice(ci * FSH + h * 128, ci * FSH + (h + 1) * 128)

                        ops_ = ps.tile([128, 128], F32, tag="ps")
                        nc.tensor.matmul(ops_, v_all[:, vsl],
                                         st["ats_all"][:, csl],
                                         start=True, stop=False,
                                         skip_group_check=True)
                        nc.tensor.matmul(ops_, S_bf, st["qs_all"][:, csl],
                                         start=False, stop=True,
                                         skip_group_check=True)
                        nc.vector.tensor_tensor(og_h[:, csl], ops_, gsil[:, csl],
                                                OP.mult)

                        sd = wk3.tile([128, 128], BF16, tag="sd")
                        nc.vector.tensor_scalar_mul(sd, ident,
                                                    st["pc_all"][:, ci : ci + 1])
                        sps = ps.tile([128, 128], F32, tag="ps")
                        nc.tensor.matmul(sps, sd, S_bf, start=True, stop=False,
                                         skip_group_check=True)
                        nc.tensor.matmul(sps, st["kw_all"][:, csl], v_all[:, vsl],
                                         start=False, stop=True,
                                         skip_group_check=True)
                        nc.vector.tensor_copy(S_bf, sps)
                    return u

                units += [chunk(ci) for ci in range(NTC)]

                if with_fin:
                    def fin():
                        og_h = st["og_h"]
                        nc.sync.dma_start(out=og_d[h * 128 : (h + 1) * 128, :],
                                          in_=og_h)
                        nc.gpsimd.collective_compute(
                            "AllGather", OP.bypass, replica_groups=GROUPS,
                            ins=[og_d[h * 128 : (h + 1) * 128, :]],
                            outs=[og_all[h][:, :]],
                        )
                    units.append(fin)
                return units

            def emit_interleaved(a, b):
                na, nb = len(a), len(b)
                ia = ib = 0
                while ia < na or ib < nb:
                    if ib < nb and (ia >= na or ib * na <= ia * nb):
                        b[ib]()
                        ib += 1
                    else:
                        a[ia]()
                        ia += 1

            with tc.tile_pool(name="hTp", bufs=1) as hTp:
                for i in range(NK):
                    tt = hTp.tile([128, L], BF16, tag=f"hT{i}")
                    nc.sync.dma_start(out=tt, in_=hT[i * 128 : (i + 1) * 128, :])
                    hT_t.append(tt)

                # ---- phase 1: v (time-major) + beta softplus ------------
                with tc.tile_pool(name="vp", bufs=1) as vp, \
                     tc.tile_pool(name="vp2", bufs=2) as vp2:
                    wv_t = []
                    for i in range(NK):
                        tt = vp.tile([128, FSH], BF16, tag=f"wv{i}")
                        nc.sync.dma_start(out=tt, in_=wv[i * 128 : (i + 1) * 128, :])
                        wv_t.append(tt)
                    wb_t = []
                    for i in range(NK):
                        tt = vp.tile([128, HL], BF16, tag=f"wb{i}")
                        nc.sync.dma_start(out=tt, in_=wb[i * 128 : (i + 1) * 128, :])
                        wb_t.append(tt)
                    bbb_t = vp.tile([128, HL], F32, tag="bbb")
                    nc.sync.dma_start(out=bbb_t, in_=bbb[:, :])

                    for ci in range(NTC):
                        csl = slice(ci * 128, (ci + 1) * 128)
                        vps = ps.tile([128, FSH], F32, tag="ps")
                        for kk in range(NK):
                            nc.tensor.matmul(vps, hT_t[kk][:, csl], wv_t[kk],
                                             start=(kk == 0), stop=(kk == NK - 1))
                        nc.scalar.activation(v_all[:, ci * FSH : (ci + 1) * FSH],
                                             vps, AF.Copy)
                        bps = ps.tile([128, HL], F32, tag="ps")
                        for kk in range(NK):
                            nc.tensor.matmul(bps, hT_t[kk][:, csl], wb_t[kk],
                                             start=(kk == 0), stop=(kk == NK - 1))
                        zb = vp2.tile([128, HL], F32, tag="zb")
                        nc.vector.tensor_tensor(zb, bps, bbb_t, OP.add)
                        bsl = slice(ci * HL, (ci + 1) * HL)
                        # softplus(z) = ln(1 + exp(z))
                        e1 = vp2.tile([128, HL], F32, tag="e1")
                        nc.scalar.activation(e1, zb, AF.Exp)
                        nc.scalar.activation(sp_all[:, bsl], e1, AF.Ln, bias=1.0)
                        e2 = vp2.tile([128, HL], F32, tag="e2")
                        nc.scalar.activation(e2, zb, AF.Exp, scale=-1.0)
                        nc.scalar.activation(spn_all[:, bsl], e2, AF.Ln, bias=1.0)

                # ---- phase 2: pipelined heads (prep+scanA braided with
                #      previous head's scanB) -----------------------------
                prev = None
                for h in range(HL):
                    pu, st = prep_units(h)
                    su = scan_b_units(h - 1, prev) if prev is not None else []
                    emit_interleaved(pu, su)
                    prev = st
            # hT pool closed: its SBUF is reused by the out-projection tiles

            # ---- tail: last head's scanB + AllGather halves + out proj --
            with tc.tile_pool(name="p4w", bufs=1) as p4w, \
                 tc.tile_pool(name="p4s", bufs=17) as p4s, \
                 tc.tile_pool(name="p4o", bufs=2) as p4o:
                lh = HL - 1
                b3 = scan_b_units(lh, prev, with_fin=False)

                def ag_half(half):
                    def u():
                        hsl2 = slice(half * (L // 2), (half + 1) * (L // 2))
                        nc.sync.dma_start(out=og_dh[half][:, :],
                                          in_=prev["og_h"][:, hsl2])
                        nc.gpsimd.collective_compute(
                            "AllGather", OP.bypass, replica_groups=GROUPS,
                            ins=[og_dh[half][:, :]],
                            outs=[og_ah[half][:, :]],
                        )
                    return u

                wo_t = []

                def wo_load():
                    for i in range(H * DV // 128):
                        tt = p4w.tile([128, FSH], BF16, tag=f"wo{i}")
                        nc.sync.dma_start(out=tt, in_=wo[i * 128 : (i + 1) * 128, :])
                        wo_t.append(tt)

                ogt = {}

                def og_src(ff, half):
                    hs, r = ff // 4, ff % 4
                    rsl = slice(r * 128, (r + 1) * 128)
                    if hs < HL - 1:
                        return og_all[hs][rsl, half * 1024 : (half + 1) * 1024]
                    return og_ah[half][rsl, :]

                def load_unit(half, ffs):
                    def u():
                        for ff in ffs:
                            s = p4s.tile([128, 1024], BF16, tag="ogs", name="ogs")
                            nc.sync.dma_start(out=s, in_=og_src(ff, half))
                            ogt[(half, ff)] = s
                    return u

                def mm_unit(b):
                    def u():
                        half = b // 2
                        coff = (b % 2) * 512
                        for tcl in range(4):
                            tci = b * 4 + tcl
                            ops_ = ps.tile([128, FSH], F32, tag="ps")
                            for ff in range(H * DV // 128):
                                nc.tensor.matmul(
                                    ops_,
                                    ogt[(half, ff)][:, coff + tcl * 128 : coff + (tcl + 1) * 128],
                                    wo_t[ff], start=(ff == 0),
                                    stop=(ff == H * DV // 128 - 1))
                            outt = p4o.tile([128, FSH], F32, tag="outt")
                            nc.scalar.activation(outt, ops_, AF.Copy)
                            nc.sync.dma_start(out=out[tci * 128 : (tci + 1) * 128, :],
                                              in_=outt)
                    return u

                nf = H * DV // 128
                hs012 = [ff for ff in range(nf) if ff // 4 < HL - 1]
                hs3 = [ff for ff in range(nf) if ff // 4 == HL - 1]
                b3aug = (b3[:8] + [ag_half(0), load_unit(0, hs3)]
                         + b3[8:] + [ag_half(1), load_unit(1, hs3)])
                emit_interleaved(
                    [wo_load, load_unit(0, hs012), load_unit(1, hs012)], b3aug)
                mm_unit(0)()
                mm_unit(1)()
                mm_unit(2)()
                mm_unit(3)()


def _wo_reordered(Wo, cols):
    """Row blocks ordered to match per-head-slot AllGather layout:
    block (hs*4 + rank) = rows of global head (4*rank + hs)."""
    blocks = [Wo[(4 * r + hs) * 128 : (4 * r + hs + 1) * 128, cols]
              for hs in range(HL) for r in range(4)]
    return np.ascontiguousarray(np.concatenate(blocks, axis=0)).astype(BF)


def prep_inputs(inputs):
    """Shard/transpose/cast full inputs into 8 per-core in_maps."""
    f = {k: np.asarray(v, dtype=np.float32) for k, v in inputs.items()}
    h = f["hidden_states"]

    hT_b = [np.ascontiguousarray(h[b].T).astype(BF) for b in range(B)]
    in_maps = []
    for c in range(NCORES):
        b, r = c // 4, c % 4
        cols = slice(r * FSH, (r + 1) * FSH)

        def convw(w):
            m = w[cols].reshape(HL, 128, K).transpose(1, 0, 2).reshape(128, HL * K)
            return np.ascontiguousarray(m).astype(np.float32)

        def convb(bias):
            return np.ascontiguousarray(bias[cols].reshape(HL, 128).T).astype(np.float32)

        m = {
            "hT": hT_b[b],
            "wq": np.ascontiguousarray(f["Wq"][:, cols]).astype(BF),
            "wk": np.ascontiguousarray(f["Wk"][:, cols]).astype(BF),
            "wg": np.ascontiguousarray(f["Wg"][:, cols]).astype(BF),
            "wv": np.ascontiguousarray(f["Wv"][:, cols]).astype(BF),
            "wb": np.ascontiguousarray(f["Wb"][:, 4 * r : 4 * r + 4]).astype(BF),
            "wo": _wo_reordered(f["Wo"], cols),
            "qcw": convw(f["qconv_w"]),
            "kcw": convw(f["kconv_w"]),
            "qcb": convb(f["qconv_b"]),
            "kcb": convb(f["kconv_b"]),
            "qnw": f["qn_w"].reshape(128, 1).astype(np.float32),
            "qnb": f["qn_b"].reshape(128, 1).astype(np.float32),
            "knw": f["kn_w"].reshape(128, 1).astype(np.float32),
            "knb": f["kn_b"].reshape(128, 1).astype(np.float32),
            "bbb": np.ascontiguousarray(
                np.broadcast_to(f["bb"][4 * r : 4 * r + 4], (128, HL))
            ).astype(np.float32),
        }
        in_maps.append(m)
    return in_maps


_NC_CACHE = {}


def get_nc():
    if "nc" not in _NC_CACHE:
        _NC_CACHE["nc"] = build_kernel()
    return _NC_CACHE["nc"]


def assemble(results):
    full = np.empty((B, L, D), np.float32)
    for c in range(NCORES):
        b, r = c // 4, c % 4
        full[b][:, r * FSH : (r + 1) * FSH] = results[c]["out"]
    return full


def kernel(**inputs) -> np.ndarray:
    nc = get_nc()
    in_maps = prep_inputs(inputs)
    res = run_bass_kernel_spmd(nc, in_maps, list(range(NCORES)))
    return assemble(res.results)


# revision 36
# speedup vs baseline: 1.1933x; 1.0923x over previous
"""GatedDeltaNet mixer on 8 Trainium2 NeuronCores.

Sharding: data-parallel over batch (cores 0-3 = batch 0, cores 4-7 = batch 1),
tensor-parallel over heads within each group (4 heads/core). Per core:
q/k/v/g/beta projections (bf16), causal depthwise conv (DVE shifted fused ops),
per-head LN (ones-matmul broadcast stats), chunked delta-rule scan (chunk=128,
decay matrices from softplus/cumsum matmuls + ACT exp), silu gating, AllGather
of the gated output within each 4-core group, then a column-sharded output
projection. Host only transposes/shards/casts inputs and concatenates outputs.
"""

import sys

sys.path.insert(0, "/opt/trn_rl_repo")

import ml_dtypes
import numpy as np

import concourse.bass as bass
import concourse.mybir as mybir
import concourse.tile as tile
from concourse.bass_utils import run_bass_kernel_spmd
from concourse.masks import make_identity, make_lower_triangular, make_upper_triangular
from concourse.tile_sem_assignment import N_PROCS
from concourse.vector_clock import ScopedClock, VectorClock

def _split_sync_waits_json(bir_json: bytes) -> bytes:
    """Legalize BIR sync waits for this container's walrus build.

    The walrus here encodes at most one sync-wait command on a regular
    instruction (two on EventSemaphore). Tile's sem-assignment attaches the
    full wait set to the consuming instruction, so spill the excess onto
    EventSemaphore carriers inserted just before it on the same engine —
    the engine executes serially, so the conjunction is preserved.
    """
    import orjson

    d = orjson.loads(bir_json)
    n = 0
    for func in d.get("functions", []):
        for bb in func.get("blocks", []):
            out = []
            for inst in bb.get("instructions", []):
                si = inst.get("sync_info")
                if si:
                    ws = si.get("on_wait") or []
                    cap = 2 if inst.get("opcode") == "EventSemaphore" else 1
                    if len(ws) > cap:
                        for w in ws[:-cap]:
                            n += 1
                            out.append({
                                "debug": inst.get("debug"),
                                "engine": inst["engine"],
                                "ins": [],
                                "name": f"SWS-{n}",
                                "opcode": "EventSemaphore",
                                "outs": [],
                                "sync_info": {"on_update": [], "on_wait": [w]},
                            })
                        si["on_wait"] = ws[-cap:]
                out.append(inst)
            bb["instructions"] = out
    return orjson.dumps(d)


def _install_wait_split_hook():
    import concourse.bass2jax as _b2j
    import concourse.bass_utils as _bu

    if getattr(_bu, "_wait_split_installed", False):
        return
    _orig = _bu.compile_bir_kernel

    def _patched(bir_json, tmpdir, neff_name="file.neff"):
        return _orig(_split_sync_waits_json(bir_json), tmpdir, neff_name)

    _bu.compile_bir_kernel = _patched
    _b2j.compile_bir_kernel = _patched
    _bu._wait_split_installed = True


_install_wait_split_hook()

BF16 = mybir.dt.bfloat16
F32 = mybir.dt.float32
AF = mybir.ActivationFunctionType
OP = mybir.AluOpType
BF = ml_dtypes.bfloat16

B, L, D = 2, 2048, 2048
H, DK, DV, K = 16, 128, 128, 4
CH = 128               # scan chunk length
NTC = L // CH          # 16 chunks
NK = D // 128          # 16 contraction tiles
HL = 4                 # heads per core
FSH = HL * DK          # 512 local feature columns
NCORES = 8
GROUPS = [[0, 1, 2, 3], [4, 5, 6, 7]]
EPS = 1e-5
NEG = -1.0e9


class _SplitDrainTC(tile.TileContext):
    """TileContext whose exit drain splits its semaphore waits.

    The walrus build here caps sync-wait commands at 1 per regular
    instruction; Tile's stock exit drain carries one wait per logical proc
    and fails to compile. Waits are moved onto a chain of NOPs instead.
    """

    def _drain_and_barrier(self, tick_clock, wait_clock):
        g = tick_clock.global_clock
        vals = [g[p] for p in range(N_PROCS)]
        for p in range(N_PROCS):
            if vals[p] <= 0:
                continue
            cvals = [vals[q] if q == p else 0 for q in range(N_PROCS)]
            d = self.nc.sync.nop(nofuse=True)
            wait_clock.add_sem_waits(d.ins, ScopedClock({None: VectorClock(cvals)}))
        self.nc.sync.drain()

        self.nc.all_engine_barrier()
        assert self.sems is not None
        popped = self.nc._tile_sem_poison_stack.pop()
        assert popped is self._sem_poison
        self.nc.clear_and_free_semaphores(list(self.sems.allocated().values()))
        self.nc.all_engine_barrier()


def build_kernel(reps: int = 1) -> bass.Bass:
    nc = bass.Bass()

    hT = nc.declare_dram_parameter("hT", [D, L], BF16, isOutput=False)
    wq = nc.declare_dram_parameter("wq", [D, FSH], BF16, isOutput=False)
    wk = nc.declare_dram_parameter("wk", [D, FSH], BF16, isOutput=False)
    wg = nc.declare_dram_parameter("wg", [D, FSH], BF16, isOutput=False)
    wv = nc.declare_dram_parameter("wv", [D, FSH], BF16, isOutput=False)
    wb = nc.declare_dram_parameter("wb", [D, HL], BF16, isOutput=False)
    wo = nc.declare_dram_parameter("wo", [H * DV, FSH], BF16, isOutput=False)
    qcw = nc.declare_dram_parameter("qcw", [128, HL * K], F32, isOutput=False)
    kcw = nc.declare_dram_parameter("kcw", [128, HL * K], F32, isOutput=False)
    qcb = nc.declare_dram_parameter("qcb", [128, HL], F32, isOutput=False)
    kcb = nc.declare_dram_parameter("kcb", [128, HL], F32, isOutput=False)
    qnw = nc.declare_dram_parameter("qnw", [128, 1], F32, isOutput=False)
    qnb = nc.declare_dram_parameter("qnb", [128, 1], F32, isOutput=False)
    knw = nc.declare_dram_parameter("knw", [128, 1], F32, isOutput=False)
    knb = nc.declare_dram_parameter("knb", [128, 1], F32, isOutput=False)
    bbb = nc.declare_dram_parameter("bbb", [128, HL], F32, isOutput=False)
    out = nc.declare_dram_parameter("out", [L, FSH], F32, isOutput=True)

    og_d = nc.dram_tensor("og_d", [FSH, L], BF16)
    og_all = [nc.dram_tensor(f"og_all{h}", [4 * 128, L], BF16) for h in range(HL)]
    og_dh = [nc.dram_tensor(f"og_dh{j}", [128, L // 2], BF16) for j in range(2)]
    og_ah = [nc.dram_tensor(f"og_ah{j}", [4 * 128, L // 2], BF16) for j in range(2)]

    with _SplitDrainTC(nc) as tc:
        with tc.tile_pool(name="ps", bufs=8, space="PSUM") as ps, \
             tc.tile_pool(name="cpool", bufs=1) as cpool:
            consts = _build_consts(nc, cpool)
            for _rep in range(reps):
                _build_main(nc, tc, (ps, ps), consts, locals())
    return nc


def _build_consts(nc, cpool):
    ones_sc = cpool.tile([128, 128], BF16, tag="ones_sc")
    nc.vector.memset(ones_sc, 1.0 / 128.0)
    ones_one = cpool.tile([128, 128], BF16, tag="ones_one")
    nc.vector.memset(ones_one, 1.0)
    negu = cpool.tile([128, 128], BF16, tag="negu")
    make_upper_triangular(nc, negu, val=-1.0, diag=True)
    ident = cpool.tile([128, 128], BF16, tag="ident")
    make_identity(nc, ident)
    maskc = cpool.tile([128, 128], BF16, tag="maskc")
    make_lower_triangular(nc, maskc, val=NEG, diag=False)
    eps_t = cpool.tile([128, 1], F32, tag="eps")
    nc.vector.memset(eps_t, EPS)
    return dict(ones_sc=ones_sc, ones_one=ones_one, negu=negu,
                ident=ident, maskc=maskc, eps_t=eps_t)


def _build_main(nc, tc, ps_pools, consts, t):
    ps, pss = ps_pools
    ones_sc, ones_one, negu = consts["ones_sc"], consts["ones_one"], consts["negu"]
    ident, maskc, eps_t = consts["ident"], consts["maskc"], consts["eps_t"]
    hT, wq, wk, wg, wv, wb = t["hT"], t["wq"], t["wk"], t["wg"], t["wv"], t["wb"]
    wo, qcw, kcw, qcb, kcb = t["wo"], t["qcw"], t["kcw"], t["qcb"], t["kcb"]
    qnw, qnb, knw, knb, bbb = t["qnw"], t["qnb"], t["knw"], t["knb"], t["bbb"]
    out, og_d, og_all = t["out"], t["og_d"], t["og_all"]
    og_dh, og_ah = t["og_dh"], t["og_ah"]

    with tc.tile_pool(name="wp", bufs=1) as wp:
        # ---- persistent smalls ------------------------------------------
        qcw_t = wp.tile([128, HL * K], F32, tag="qcw")
        nc.sync.dma_start(out=qcw_t, in_=qcw[:, :])
        kcw_t = wp.tile([128, HL * K], F32, tag="kcw")
        nc.sync.dma_start(out=kcw_t, in_=kcw[:, :])
        qcb_t = wp.tile([128, HL], F32, tag="qcb")
        nc.sync.dma_start(out=qcb_t, in_=qcb[:, :])
        kcb_t = wp.tile([128, HL], F32, tag="kcb")
        nc.sync.dma_start(out=kcb_t, in_=kcb[:, :])
        qnw_t = wp.tile([128, 1], F32, tag="qnw")
        nc.sync.dma_start(out=qnw_t, in_=qnw[:, :])
        qnb_t = wp.tile([128, 1], F32, tag="qnb")
        nc.sync.dma_start(out=qnb_t, in_=qnb[:, :])
        knw_t = wp.tile([128, 1], F32, tag="knw")
        nc.sync.dma_start(out=knw_t, in_=knw[:, :])
        knb_t = wp.tile([128, 1], F32, tag="knb")
        nc.sync.dma_start(out=knb_t, in_=knb[:, :])

        v_all = wp.tile([128, NTC * FSH], BF16, tag="v_all")
        sp_all = wp.tile([128, NTC * HL], F32, tag="sp_all")
        spn_all = wp.tile([128, NTC * HL], F32, tag="spn_all")

        hT_t = []  # populated inside the hT pool scope below

        with tc.tile_pool(name="wk1", bufs=2) as wk1, \
             tc.tile_pool(name="wk2", bufs=2) as wk2, \
             tc.tile_pool(name="wk3", bufs=3) as wk3, \
             tc.tile_pool(name="wk4", bufs=1) as wk4, \
             tc.tile_pool(name="wst", bufs=3) as wst:

            def prep_units(h):
                hsl = slice(h * 128, (h + 1) * 128)
                st = {}
                units = []

                def start_tensor(wparam, name):
                    def u():
                        wt = wst.tile([128, NK, 128], BF16, tag="wst")
                        nc.sync.dma_start(
                            out=wt,
                            in_=wparam[:, hsl].rearrange("(a p) b -> p a b", p=128))
                        st[name + "_w"] = wt
                        if name in ("q", "k"):
                            xpad = wk4.tile([128, 4 + L], BF16, tag="xpad")
                            nc.vector.memset(xpad[:, 0:4], 0.0)
                            st[name + "_xpad"] = xpad
                    return u

                def proj_tile(name, tt2, sink):
                    def u():
                        wt = st[name + "_w"]
                        pps = ps.tile([128, 512], F32, tag="ps")
                        tsl = slice(tt2 * 512, (tt2 + 1) * 512)
                        for kk in range(NK):
                            nc.tensor.matmul(pps, wt[:, kk, :], hT_t[kk][:, tsl],
                                             start=(kk == 0), stop=(kk == NK - 1))
                        sink(tt2, pps)
                    return u

                def xpad_sink(name):
                    def sink(tt2, pps):
                        xpad = st[name + "_xpad"]
                        nc.scalar.activation(
                            xpad[:, 4 + tt2 * 512 : 4 + (tt2 + 1) * 512],
                            pps, AF.Copy)
                    return sink

                def gsil_sink(tt2, pps):
                    if "gsil" not in st:
                        st["gsil"] = wk1.tile([128, L], BF16, tag="gsil", name="gsil")
                    nc.scalar.activation(
                        st["gsil"][:, tt2 * 512 : (tt2 + 1) * 512], pps, AF.Silu)

                def conv_unit(name, cw, cb):
                    def u():
                        xpad = st[name + "_xpad"]
                        y = wk2.tile([128, L], BF16, tag="convy")
                        nc.vector.tensor_scalar_mul(y, xpad[:, 1 : 1 + L],
                                                    cw[:, h * K : h * K + 1])
                        for s in (1, 2, 3):
                            nc.vector.scalar_tensor_tensor(
                                y, xpad[:, 1 + s : 1 + s + L],
                                cw[:, h * K + s : h * K + s + 1], y,
                                OP.mult, OP.add)
                        nc.scalar.activation(y, y, AF.Silu, bias=cb[:, h : h + 1])
                        st[name + "_sil"] = y
                    return u

                def ln_unit(name, dstname, tt2, lw, lb):
                    def u():
                        sil = st[name + "_sil"]
                        if dstname not in st:
                            st[dstname] = wk1.tile([128, L], BF16, tag=dstname,
                                                   name=dstname)
                        dst = st[dstname]
                        tsl = slice(tt2 * 512, (tt2 + 1) * 512)
                        sq = wk4.tile([128, 512], BF16, tag="sq")
                        nc.scalar.square(sq, sil[:, tsl])
                        mups = ps.tile([128, 512], F32, tag="ps")
                        nc.tensor.matmul(mups, ones_sc, sil[:, tsl],
                                         start=True, stop=True)
                        sqps = ps.tile([128, 512], F32, tag="ps")
                        nc.tensor.matmul(sqps, ones_sc, sq, start=True, stop=True)
                        m2 = wk4.tile([128, 512], F32, tag="m2")
                        nc.scalar.square(m2, mups)
                        vt = wk4.tile([128, 512], F32, tag="vt")
                        nc.vector.tensor_tensor(vt, sqps, m2, OP.subtract)
                        nc.scalar.activation(vt, vt, AF.Ln, bias=eps_t)
                        r0 = wk2.tile([128, 512], BF16, tag="r0")
                        nc.scalar.activation(r0, vt, AF.Exp, scale=-0.5)
                        r1 = wk2.tile([128, 512], BF16, tag="r1")
                        nc.vector.tensor_scalar_mul(r1, r0, lw)
                        s1 = wk2.tile([128, 512], BF16, tag="s1")
                        nc.vector.scalar_tensor_tensor(s1, mups, -1.0, r1,
                                                       OP.mult, OP.mult)
                        t1 = wk2.tile([128, 512], BF16, tag="t1")
                        nc.vector.tensor_tensor(t1, sil[:, tsl], r1, OP.mult)
                        nc.vector.scalar_tensor_tensor(dst[:, tsl], t1, lb, s1,
                                                       OP.add, OP.add)
                    return u

                def trans_unit():
                    # PE-mode transpose: dma_start_transpose would flip the
                    # DMA xbar mode, which Tile serializes against the
                    # collectives -- stalling every head behind the previous
                    # head's AllGather.
                    kln = st["kln"]
                    ktm = wk1.tile([128, L], BF16, tag="klntm")
                    for ci in range(NTC):
                        csl = slice(ci * 128, (ci + 1) * 128)
                        tps = ps.tile([128, 128], BF16, tag="ps")
                        nc.tensor.transpose(tps, kln[:, csl], ident)
                        nc.scalar.activation(ktm[:, csl], tps, AF.Copy)
                    st["kln_tm"] = ktm
                    sb = wk1.tile([128, 128], BF16, tag="sbf")
                    nc.vector.memset(sb, 0.0)
                    st["S_bf"] = sb
                    st["og_h"] = wk1.tile([128, L], BF16, tag="ogh", name="ogh")
                    st["ats_all"] = wk1.tile([128, L], BF16, tag="ats_all",
                                             name="ats_all")
                    st["qs_all"] = wk1.tile([128, L], BF16, tag="qs_all",
                                            name="qs_all")
                    st["kw_all"] = wk1.tile([128, L], BF16, tag="kw_all",
                                            name="kw_all")
                    st["pc_all"] = wk1.tile([128, NTC], F32, tag="pc_all",
                                            name="pc_all")

                units.append(start_tensor(wq, "q"))
                units += [proj_tile("q", t2, xpad_sink("q")) for t2 in range(4)]
                units.append(conv_unit("q", qcw_t, qcb_t))
                units.append(start_tensor(wk, "k"))
                units += [proj_tile("k", t2, xpad_sink("k")) for t2 in range(4)]
                units.append(conv_unit("k", kcw_t, kcb_t))
                units.append(start_tensor(wg, "g"))
                units += [proj_tile("g", t2, gsil_sink) for t2 in range(4)]
                units += [ln_unit("q", "qln", t2, qnw_t, qnb_t) for t2 in range(4)]
                units += [ln_unit("k", "kln", t2, knw_t, knb_t) for t2 in range(4)]
                units.append(trans_unit)
                units += [scan_a_chunk(h, st, ci) for ci in range(NTC)]
                return units, st

            def scan_a_chunk(h, st, ci):
                    def u():
                        qln, kln, kln_tm = st["qln"], st["kln"], st["kln_tm"]
                        csl = slice(ci * 128, (ci + 1) * 128)
                        spc = sp_all[:, ci * HL + h : ci * HL + h + 1]
                        spnc = spn_all[:, ci * HL + h : ci * HL + h + 1]

                        Yt = wk3.tile([128, 128], BF16, tag="Y")
                        nc.vector.tensor_scalar_mul(Yt, negu, spc)
                        # separate psums for the unmasked/masked cumsums:
                        # accumulating the mask into the same bank the ptile
                        # exp just read serializes PE behind ACT on that bank
                        gps = ps.tile([128, 128], F32, tag="ps")
                        nc.tensor.matmul(gps, ones_one, Yt, start=True, stop=True,
                                         skip_group_check=True)
                        gmps = ps.tile([128, 128], F32, tag="ps")
                        nc.tensor.matmul(gmps, ones_one, Yt, start=True, stop=False,
                                         skip_group_check=True)
                        nc.tensor.matmul(gmps, ident, maskc, start=False, stop=True,
                                         skip_group_check=True)
                        gtps = ps.tile([128, 128], F32, tag="ps")
                        nc.tensor.matmul(gtps, Yt, ones_one, start=True, stop=True,
                                         skip_group_check=True)
                        ptile = wk3.tile([128, 128], F32, tag="pt")
                        nc.scalar.activation(ptile, gps, AF.Exp)
                        scol = wk3.tile([128, 1], F32, tag="scol")
                        nc.vector.scalar_tensor_tensor(scol, spnc, -1.0,
                                                       gtps[:, 0:1],
                                                       OP.mult, OP.subtract)
                        dexp = wk3.tile([128, 128], F32, tag="dexp")
                        nc.scalar.activation(dexp, gmps, AF.Exp, bias=scol)

                        atps = ps.tile([128, 128], F32, tag="ps")
                        nc.tensor.matmul(atps, kln[:, csl], qln[:, csl],
                                         start=True, stop=True)
                        nc.vector.tensor_tensor(st["ats_all"][:, csl], atps, dexp,
                                                OP.mult)
                        nc.vector.tensor_tensor(st["qs_all"][:, csl], qln[:, csl],
                                                ptile, OP.mult)
                        nc.vector.tensor_scalar_mul(st["kw_all"][:, csl],
                                                    kln_tm[:, csl],
                                                    dexp[:, 127:128])
                        nc.vector.tensor_copy(st["pc_all"][:, ci : ci + 1],
                                              ptile[:, 127:128])
                    return u

            def scan_b_units(h, st, with_fin=True):
                """Serial state recurrence + gated output (no ACT work)."""
                units = []

                def chunk(ci):
                    def u():
                        S_bf, og_h, gsil = st["S_bf"], st["og_h"], st["gsil"]
                        csl = slice(ci * 128, (ci + 1) * 128)
                        vsl = slice(ci * FSH + h * 128, ci * FSH + (h + 1) * 128)

                        ops_ = ps.tile([128, 128], F32, tag="ps")
                        nc.tensor.matmul(ops_, v_all[:, vsl],
                                         st["ats_all"][:, csl],
                                         start=True, stop=False,
                                         skip_group_check=True)
                        nc.tensor.matmul(ops_, S_bf, st["qs_all"][:, csl],
                                         start=False, stop=True,
                                         skip_group_check=True)
                        nc.vector.tensor_tensor(og_h[:, csl], ops_, gsil[:, csl],
                                                OP.mult)

                        sd = wk3.tile([128, 128], BF16, tag="sd")
                        nc.vector.tensor_scalar_mul(sd, ident,
                                                    st["pc_all"][:, ci : ci + 1])
                        sps = ps.tile([128, 128], F32, tag="ps")
                        nc.tensor.matmul(sps, sd, S_bf, start=True, stop=False,
                                         skip_group_check=True)
                        nc.tensor.matmul(sps, st["kw_all"][:, csl], v_all[:, vsl],
                                         start=False, stop=True,
                                         skip_group_check=True)
                        nc.vector.tensor_copy(S_bf, sps)
                    return u

                units += [chunk(ci) for ci in range(NTC)]

                if with_fin:
                    def fin():
                        og_h = st["og_h"]
                        nc.sync.dma_start(out=og_d[h * 128 : (h + 1) * 128, :],
                                          in_=og_h)
                        nc.gpsimd.collective_compute(
                            "AllGather", OP.bypass, replica_groups=GROUPS,
                            ins=[og_d[h * 128 : (h + 1) * 128, :]],
                            outs=[og_all[h][:, :]],
                        )
                    units.append(fin)
                return units

            def emit_interleaved(a, b):
                na, nb = len(a), len(b)
                ia = ib = 0
                while ia < na or ib < nb:
                    if ib < nb and (ia >= na or ib * na <= ia * nb):
                        b[ib]()
                        ib += 1
                    else:
                        a[ia]()
                        ia += 1

            with tc.tile_pool(name="hTp", bufs=1) as hTp:
                # ---- phase 1: v (time-major) + beta softplus ------------
                # interleave the small wv/wb loads with the 8 MB hT load so
                # the first K-tile's matmuls start ~2 us in instead of
                # waiting for the whole load train on the SP queue
                with tc.tile_pool(name="vp", bufs=1) as vp, \
                     tc.tile_pool(name="vp2", bufs=2) as vp2:
                    wv_t, wb_t = [], []
                    bbb_t = vp.tile([128, HL], F32, tag="bbb")
                    nc.sync.dma_start(out=bbb_t, in_=bbb[:, :])
                    for i in range(NK):
                        tt = vp.tile([128, FSH], BF16, tag=f"wv{i}")
                        nc.sync.dma_start(out=tt, in_=wv[i * 128 : (i + 1) * 128, :])
                        wv_t.append(tt)
                        tt = vp.tile([128, HL], BF16, tag=f"wb{i}")
                        nc.sync.dma_start(out=tt, in_=wb[i * 128 : (i + 1) * 128, :])
                        wb_t.append(tt)
                        ht = hTp.tile([128, L], BF16, tag=f"hT{i}")
                        nc.sync.dma_start(out=ht, in_=hT[i * 128 : (i + 1) * 128, :])
                        hT_t.append(ht)

                    for ci in range(NTC):
                        csl = slice(ci * 128, (ci + 1) * 128)
                        vps = ps.tile([128, FSH], F32, tag="ps")
                        for kk in range(NK):
                            nc.tensor.matmul(vps, hT_t[kk][:, csl], wv_t[kk],
                                             start=(kk == 0), stop=(kk == NK - 1))
                        nc.scalar.activation(v_all[:, ci * FSH : (ci + 1) * FSH],
                                             vps, AF.Copy)
                        bps = ps.tile([128, HL], F32, tag="ps")
                        for kk in range(NK):
                            nc.tensor.matmul(bps, hT_t[kk][:, csl], wb_t[kk],
                                             start=(kk == 0), stop=(kk == NK - 1))
                        zb = vp2.tile([128, HL], F32, tag="zb")
                        nc.vector.tensor_tensor(zb, bps, bbb_t, OP.add)
                        bsl = slice(ci * HL, (ci + 1) * HL)
                        # softplus(z) = ln(1 + exp(z))
                        e1 = vp2.tile([128, HL], F32, tag="e1")
                        nc.scalar.activation(e1, zb, AF.Exp)
                        nc.scalar.activation(sp_all[:, bsl], e1, AF.Ln, bias=1.0)
                        e2 = vp2.tile([128, HL], F32, tag="e2")
                        nc.scalar.activation(e2, zb, AF.Exp, scale=-1.0)
                        nc.scalar.activation(spn_all[:, bsl], e2, AF.Ln, bias=1.0)

                # ---- phase 2: pipelined heads (prep+scanA braided with
                #      previous head's scanB) -----------------------------
                prev = None
                for h in range(HL):
                    pu, st = prep_units(h)
                    su = scan_b_units(h - 1, prev) if prev is not None else []
                    emit_interleaved(pu, su)
                    prev = st
            # hT pool closed: its SBUF is reused by the out-projection tiles

            # ---- tail: last head's scanB + AllGather halves + out proj --
            with tc.tile_pool(name="p4w", bufs=1) as p4w, \
                 tc.tile_pool(name="p4s", bufs=17) as p4s, \
                 tc.tile_pool(name="p4o", bufs=2) as p4o:
                lh = HL - 1
                b3 = scan_b_units(lh, prev, with_fin=False)

                def ag_half(half):
                    def u():
                        hsl2 = slice(half * (L // 2), (half + 1) * (L // 2))
                        nc.sync.dma_start(out=og_dh[half][:, :],
                                          in_=prev["og_h"][:, hsl2])
                        nc.gpsimd.collective_compute(
                            "AllGather", OP.bypass, replica_groups=GROUPS,
                            ins=[og_dh[half][:, :]],
                            outs=[og_ah[half][:, :]],
                        )
                    return u

                wo_t = []

                def wo_load():
                    for i in range(H * DV // 128):
                        tt = p4w.tile([128, FSH], BF16, tag=f"wo{i}")
                        nc.sync.dma_start(out=tt, in_=wo[i * 128 : (i + 1) * 128, :])
                        wo_t.append(tt)

                ogt = {}

                def og_src(ff, half):
                    hs, r = ff // 4, ff % 4
                    rsl = slice(r * 128, (r + 1) * 128)
                    if hs < HL - 1:
                        return og_all[hs][rsl, half * 1024 : (half + 1) * 1024]
                    return og_ah[half][rsl, :]

                def load_unit(half, ffs):
                    def u():
                        for ff in ffs:
                            s = p4s.tile([128, 1024], BF16, tag="ogs", name="ogs")
                            nc.sync.dma_start(out=s, in_=og_src(ff, half))
                            ogt[(half, ff)] = s
                    return u

                def mm_unit(b):
                    def u():
                        half = b // 2
                        coff = (b % 2) * 512
                        for tcl in range(4):
                            tci = b * 4 + tcl
                            ops_ = ps.tile([128, FSH], F32, tag="ps")
                            for ff in range(H * DV // 128):
                                nc.tensor.matmul(
                                    ops_,
                                    ogt[(half, ff)][:, coff + tcl * 128 : coff + (tcl + 1) * 128],
                                    wo_t[ff], start=(ff == 0),
                                    stop=(ff == H * DV // 128 - 1))
                            outt = p4o.tile([128, FSH], F32, tag="outt")
                            nc.scalar.activation(outt, ops_, AF.Copy)
                            nc.sync.dma_start(out=out[tci * 128 : (tci + 1) * 128, :],
                                              in_=outt)
                    return u

                nf = H * DV // 128
                hs012 = [ff for ff in range(nf) if ff // 4 < HL - 1]
                hs3 = [ff for ff in range(nf) if ff // 4 == HL - 1]
                b3aug = (b3[:8] + [ag_half(0), load_unit(0, hs3)]
                         + b3[8:] + [ag_half(1), load_unit(1, hs3)])
                emit_interleaved(
                    [wo_load, load_unit(0, hs012), load_unit(1, hs012)], b3aug)
                mm_unit(0)()
                mm_unit(1)()
                mm_unit(2)()
                mm_unit(3)()


def _wo_reordered(Wo, cols):
    """Row blocks ordered to match per-head-slot AllGather layout:
    block (hs*4 + rank) = rows of global head (4*rank + hs)."""
    blocks = [Wo[(4 * r + hs) * 128 : (4 * r + hs + 1) * 128, cols]
              for hs in range(HL) for r in range(4)]
    return np.ascontiguousarray(np.concatenate(blocks, axis=0)).astype(BF)


def prep_inputs(inputs):
    """Shard/transpose/cast full inputs into 8 per-core in_maps."""
    f = {k: np.asarray(v, dtype=np.float32) for k, v in inputs.items()}
    h = f["hidden_states"]

    hT_b = [np.ascontiguousarray(h[b].T).astype(BF) for b in range(B)]
    in_maps = []
    for c in range(NCORES):
        b, r = c // 4, c % 4
        cols = slice(r * FSH, (r + 1) * FSH)

        def convw(w):
            m = w[cols].reshape(HL, 128, K).transpose(1, 0, 2).reshape(128, HL * K)
            return np.ascontiguousarray(m).astype(np.float32)

        def convb(bias):
            return np.ascontiguousarray(bias[cols].reshape(HL, 128).T).astype(np.float32)

        m = {
            "hT": hT_b[b],
            "wq": np.ascontiguousarray(f["Wq"][:, cols]).astype(BF),
            "wk": np.ascontiguousarray(f["Wk"][:, cols]).astype(BF),
            "wg": np.ascontiguousarray(f["Wg"][:, cols]).astype(BF),
            "wv": np.ascontiguousarray(f["Wv"][:, cols]).astype(BF),
            "wb": np.ascontiguousarray(f["Wb"][:, 4 * r : 4 * r + 4]).astype(BF),
            "wo": _wo_reordered(f["Wo"], cols),
            "qcw": convw(f["qconv_w"]),
            "kcw": convw(f["kconv_w"]),
            "qcb": convb(f["qconv_b"]),
            "kcb": convb(f["kconv_b"]),
            "qnw": f["qn_w"].reshape(128, 1).astype(np.float32),
            "qnb": f["qn_b"].reshape(128, 1).astype(np.float32),
            "knw": f["kn_w"].reshape(128, 1).astype(np.float32),
            "knb": f["kn_b"].reshape(128, 1).astype(np.float32),
            "bbb": np.ascontiguousarray(
                np.broadcast_to(f["bb"][4 * r : 4 * r + 4], (128, HL))
            ).astype(np.float32),
        }
        in_maps.append(m)
    return in_maps


_NC_CACHE = {}


def get_nc():
    if "nc" not in _NC_CACHE:
        _NC_CACHE["nc"] = build_kernel()
    return _NC_CACHE["nc"]


def assemble(results):
    full = np.empty((B, L, D), np.float32)
    for c in range(NCORES):
        b, r = c // 4, c % 4
        full[b][:, r * FSH : (r + 1) * FSH] = results[c]["out"]
    return full


def kernel(**inputs) -> np.ndarray:
    nc = get_nc()
    in_maps = prep_inputs(inputs)
    res = run_bass_kernel_spmd(nc, in_maps, list(range(NCORES)))
    return assemble(res.results)
